# Initial kernel scaffold
#
"""3-layer GCN encoder on 8 Trainium2 NeuronCores (Bass/Tile).

Sharding: 1D node partition (contiguous ranges) across 8 cores.
Per layer: local z = input @ W, AllGather z -> full node-feature table,
then each core aggregates messages for its own nodes via
indirect-DMA gathers + one-hot matmul segment sums on the TensorEngine.
"""
import math

import numpy as np
import ml_dtypes

BF16 = ml_dtypes.bfloat16

# problem constants (hardcoded per contract)
N = 100000
E = 1600000
IN_DIM = 512
F = 256
LN_EPS = 1e-5
NCORES = 8
P = 128


# ---------------------------------------------------------------- host side


def _preprocess(edge_index, n_pad, npc, nt):
    """Sort/bin edges by dst, pack per-core edge-chunk metadata.

    Returns (K, per-core list of dicts) where every packed array has an
    identical shape across cores (SPMD requirement).
    """
    src = np.ascontiguousarray(edge_index[0]).astype(np.int64)
    dst = np.ascontiguousarray(edge_index[1]).astype(np.int64)

    deg = 1.0 + np.bincount(dst, minlength=n_pad).astype(np.float64)
    dis = 1.0 / np.sqrt(deg)
    norm = (dis[src] * dis[dst]).astype(np.float32)
    invdeg = (1.0 / deg).astype(np.float32)

    core = dst // npc
    percore = []
    for r in range(NCORES):
        m = core == r
        d_r = dst[m]
        o = np.argsort(d_r, kind="stable")
        s_s = src[m][o]
        d_s = d_r[o]
        w_s = norm[m][o]
        tid = (d_s - r * npc) >> 7
        cnt = np.bincount(tid, minlength=nt)
        starts = np.concatenate([[0], np.cumsum(cnt)])
        pos = np.arange(len(d_s)) - starts[tid]
        percore.append((s_s, d_s, w_s, tid, pos, cnt))

    kmax = max(int(math.ceil(pc[5].max() / P)) for pc in percore)
    kmax = max(kmax, 1)

    out = []
    for r in range(NCORES):
        s_s, d_s, w_s, tid, pos, cnt = percore[r]
        si = np.zeros((nt, kmax * P), np.int32)
        dl = np.zeros((nt, kmax * P), np.float32)
        nm = np.zeros((nt, kmax * P), np.float32)
        si[tid, pos] = s_s.astype(np.int32)
        dl[tid, pos] = (d_s % P).astype(np.float32)
        nm[tid, pos] = w_s

        def tr(a):
            return np.ascontiguousarray(
                a.reshape(nt, kmax, P).transpose(2, 0, 1).reshape(P, nt * kmax)
            )

        iv = np.ascontiguousarray(
            invdeg[r * npc:(r + 1) * npc].astype(np.float32).reshape(nt, P).T
        )
        out.append(
            dict(
                src_idx=tr(si),
                dstloc=tr(dl).astype(BF16),
                norm=tr(nm).astype(BF16),
                invdeg=iv,
            )
        )
    return kmax, out


# ---------------------------------------------------------------- device side


def _build_program(npc, nt, kk, in_dim, f, ncores):
    """Build the 3-layer GCN SPMD program. Returns compiled Bacc."""
    from concourse import bass, mybir, tile, bacc
    from concourse.masks import make_identity

    bf = mybir.dt.bfloat16
    f32 = mybir.dt.float32
    AF = mybir.ActivationFunctionType
    Alu = mybir.AluOpType

    nc = bacc.Bacc("TRN2", target_bir_lowering=False, debug=False,
                   enable_asserts=True, num_devices=ncores)

    # ---- I/O
    x_in = nc.dram_tensor("x", [npc, in_dim], bf, kind="ExternalInput")
    w_in = [
        nc.dram_tensor("w1", [in_dim, f], bf, kind="ExternalInput"),
        nc.dram_tensor("w2", [f, f], bf, kind="ExternalInput"),
        nc.dram_tensor("w3", [f, f], bf, kind="ExternalInput"),
    ]
    srcidx_in = nc.dram_tensor("src_idx", [P, nt * kk], mybir.dt.int32,
                               kind="ExternalInput")
    dstloc_in = nc.dram_tensor("dstloc", [P, nt * kk], bf, kind="ExternalInput")
    norm_in = nc.dram_tensor("norm", [P, nt * kk], bf, kind="ExternalInput")
    invdeg_in = nc.dram_tensor("invdeg", [P, nt], f32, kind="ExternalInput")
    y_out = nc.dram_tensor("y", [npc, f], f32, kind="ExternalOutput")

    # ---- internal DRAM
    ag_in = nc.dram_tensor("ag_in", [npc, f], bf, kind="Internal")
    h_full = nc.dram_tensor("h_full", [npc * ncores, f], bf, kind="Internal")
    zd_loc = nc.dram_tensor("zd_loc", [npc, f], bf, kind="Internal")
    out_loc = nc.dram_tensor("out_loc", [npc, f], bf, kind="Internal")

    with tile.TileContext(nc) as tc:
        with tc.tile_pool(name="consts", bufs=1) as cpool, \
             tc.tile_pool(name="work", bufs=3) as wpool, \
             tc.tile_pool(name="gather", bufs=3) as gpool, \
             tc.tile_pool(name="stats", bufs=8) as spool, \
             tc.tile_pool(name="psA", bufs=2, space="PSUM") as psA, \
             tc.tile_pool(name="psB", bufs=4, space="PSUM") as psB:

            # ---- persistent constants in SBUF
            srcidx_sb = cpool.tile([P, nt * kk], mybir.dt.int32, tag="srcidx")
            nc.sync.dma_start(srcidx_sb[:], srcidx_in[:])
            dstloc_sb = cpool.tile([P, nt * kk], bf, tag="dstloc")
            nc.sync.dma_start(dstloc_sb[:], dstloc_in[:])
            norm_sb = cpool.tile([P, nt * kk], bf, tag="norm")
            nc.sync.dma_start(norm_sb[:], norm_in[:])
            invdeg_sb = cpool.tile([P, nt], f32, tag="invdeg")
            nc.sync.dma_start(invdeg_sb[:], invdeg_in[:])

            # weight tiles: w[l] as list of [P, f] k-blocks
            w_sb = []
            for l, w in enumerate(w_in):
                kin = w.shape[0]
                blocks = []
                for b in range(kin // P):
                    t = cpool.tile([P, f], bf, tag=f"w{l}_{b}")
                    nc.sync.dma_start(t[:], w[b * P:(b + 1) * P, :])
                    blocks.append(t)
                w_sb.append(blocks)

            # iota along free (value = position within 128-block), bf16 exact
            iota_sb = cpool.tile([P, kk * P], bf, tag="iota")
            nc.gpsimd.iota(iota_sb[:], pattern=[[0, kk], [1, P]], base=0,
                           channel_multiplier=0,
                           allow_small_or_imprecise_dtypes=True)
            # identity for the self-loop matmul
            ident_sb = cpool.tile([P, P], bf, tag="ident")
            make_identity(nc, ident_sb[:])

            layer_in = [x_in, out_loc, out_loc]
            kin_l = [in_dim, f, f]

            for l in range(3):
                # ---------- phase A: z = input @ W (local rows)
                for t in range(nt):
                    zp = psA.tile([P, f], f32, tag="zpsum")
                    nkb = kin_l[l] // P
                    for kb in range(nkb):
                        xT = wpool.tile([P, P], bf, tag="xT")
                        nc.sync.dma_start(
                            xT[:],
                            layer_in[l][t * P:(t + 1) * P, kb * P:(kb + 1) * P],
                            transpose=True,
                        )
                        nc.tensor.matmul(out=zp[:], lhsT=xT[:], rhs=w_sb[l][kb][:],
                                         start=(kb == 0), stop=(kb == nkb - 1))
                    zt = wpool.tile([P, f], bf, tag="zt")
                    nc.scalar.copy(zt[:], zp[:])
                    nc.sync.dma_start(ag_in[t * P:(t + 1) * P, :], zt[:])
                    zdt = wpool.tile([P, f], bf, tag="zdt")
                    nc.vector.tensor_scalar(zdt[:], zp[:], invdeg_sb[:, t:t + 1],
                                            None, op0=Alu.mult)
                    nc.sync.dma_start(zd_loc[t * P:(t + 1) * P, :], zdt[:])

                # ---------- AllGather z -> h_full
                nc.gpsimd.collective_compute(
                    "AllGather",
                    Alu.bypass,
                    replica_groups=[list(range(ncores))],
                    ins=[ag_in[:]],
                    outs=[h_full[:]],
                )

                # ---------- phase B: aggregate + self + relu (+ LN)
                for t in range(nt):
                    g_t = gpool.tile([P, kk * f], bf, tag="gt")
                    nc.gpsimd.indirect_dma_start(
                        out=g_t[:],
                        out_offset=None,
                        in_=h_full[:],
                        in_offset=bass.IndirectOffsetOnAxis(
                            ap=srcidx_sb[:, t * kk:(t + 1) * kk], axis=0),
                    )
                    s_t = gpool.tile([P, kk * P], bf, tag="st")
                    dl3 = dstloc_sb[:, t * kk:(t + 1) * kk].to_broadcast(
                        [P, kk, P])
                    nm3 = norm_sb[:, t * kk:(t + 1) * kk].to_broadcast(
                        [P, kk, P])
                    s3 = s_t[:].rearrange("p (k q) -> p k q", q=P)
                    i3 = iota_sb[:].rearrange("p (k q) -> p k q", q=P)
                    nc.vector.tensor_tensor(s3, i3, dl3, op=Alu.is_equal)
                    nc.vector.tensor_tensor(s3, s3, nm3, op=Alu.mult)

                    ap = psB.tile([P, f], f32, tag="agg")
                    for j in range(kk):
                        nc.tensor.matmul(out=ap[:],
                                         lhsT=s_t[:, j * P:(j + 1) * P],
                                         rhs=g_t[:, j * f:(j + 1) * f],
                                         start=(j == 0), stop=False)
                    zdt = wpool.tile([P, f], bf, tag="zdl")
                    nc.sync.dma_start(zdt[:], zd_loc[t * P:(t + 1) * P, :])
                    nc.tensor.matmul(out=ap[:], lhsT=ident_sb[:], rhs=zdt[:],
                                     start=False, stop=True)

                    if l < 2:
                        # relu + layernorm (g=1, be=0 for this model)
                        vr = wpool.tile([P, f], f32, tag="vr")
                        musum = spool.tile([P, 1], f32, tag="musum")
                        nc.scalar.activation(vr[:], ap[:], AF.Relu,
                                             accum_out=musum[:])
                        mu = spool.tile([P, 1], f32, tag="mu")
                        nc.scalar.activation(mu[:], musum[:], AF.Copy,
                                             scale=1.0 / f)
                        d = wpool.tile([P, f], f32, tag="d")
                        nc.vector.tensor_scalar(d[:], vr[:], mu[:], None,
                                                op0=Alu.subtract)
                        sq = wpool.tile([P, f], f32, tag="sq")
                        varsum = spool.tile([P, 1], f32, tag="varsum")
                        nc.scalar.activation(sq[:], d[:], AF.Square,
                                             accum_out=varsum[:])
                        std = spool.tile([P, 1], f32, tag="std")
                        nc.scalar.activation(std[:], varsum[:], AF.Sqrt,
                                             bias=LN_EPS, scale=1.0 / f)
                        rs = spool.tile([P, 1], f32, tag="rs")
                        nc.vector.reciprocal(rs[:], std[:])
                        y_t = wpool.tile([P, f], bf, tag="yt")
                        nc.vector.tensor_scalar(y_t[:], d[:], rs[:], None,
                                                op0=Alu.mult)
                        nc.sync.dma_start(out_loc[t * P:(t + 1) * P, :], y_t[:])
                    else:
                        y_t = wpool.tile([P, f], f32, tag="yf")
                        nc.scalar.activation(y_t[:], ap[:], AF.Relu)
                        nc.sync.dma_start(y_out[t * P:(t + 1) * P, :], y_t[:])

    nc.compile()
    return nc


# ---------------------------------------------------------------- entry point


def run_gcn(x, edge_index, Ws, n, e, in_dim, f, ncores=NCORES, trace=False):
    """Generic runner used by kernel() and by the mini test."""
    from concourse import bass_utils

    npc = int(math.ceil(n / (ncores * P))) * P
    n_pad = npc * ncores
    nt = npc // P

    kmax, meta = _preprocess(edge_index, n_pad, npc, nt)

    x_pad = np.zeros((n_pad, in_dim), np.float32)
    x_pad[:n] = np.asarray(x, np.float32)
    x_bf = x_pad.astype(BF16)
    w_bf = [np.asarray(w, np.float32).astype(BF16) for w in Ws]

    nc = _build_program(npc, nt, kmax, in_dim, f, ncores)

    in_maps = []
    for r in range(ncores):
        m = meta[r]
        in_maps.append({
            "x": x_bf[r * npc:(r + 1) * npc],
            "w1": w_bf[0], "w2": w_bf[1], "w3": w_bf[2],
            "src_idx": m["src_idx"],
            "dstloc": m["dstloc"],
            "norm": m["norm"],
            "invdeg": m["invdeg"],
        })

    res = bass_utils.run_bass_kernel_spmd(
        nc, in_maps, core_ids=list(range(ncores)), trace=trace)

    y = np.concatenate([res.results[r]["y"] for r in range(ncores)], axis=0)
    return y[:n], res


def kernel(x, edge_index, W1, b1, W2, b2, W3, b3, g1, be1, g2, be2):
    # b1..b3 are zeros and g/be are identity for this model; verified on host
    # (they fold away from the device program).
    assert not np.any(np.asarray(b1)) and not np.any(np.asarray(b2)) \
        and not np.any(np.asarray(b3))
    assert np.all(np.asarray(g1) == 1) and np.all(np.asarray(g2) == 1)
    assert not np.any(np.asarray(be1)) and not np.any(np.asarray(be2))

    y, _ = run_gcn(np.asarray(x), np.asarray(edge_index), [W1, W2, W3],
                   N, E, IN_DIM, F)
    return y


# revision 3
# speedup vs baseline: 33.7022x; 33.7022x over previous
"""3-layer GCN encoder on 8 Trainium2 NeuronCores (Bass/Tile).

Sharding: 1D node partition (contiguous ranges) across 8 cores.
Per layer: local z = input @ W, AllGather z -> full node-feature table,
then each core aggregates messages for its own nodes via
indirect-DMA gathers + one-hot matmul segment sums on the TensorEngine.
"""
import math

import numpy as np
import ml_dtypes

BF16 = ml_dtypes.bfloat16

# problem constants (hardcoded per contract)
N = 100000
E = 1600000
IN_DIM = 512
F = 256
LN_EPS = 1e-5
NCORES = 8
P = 128


# ---------------------------------------------------------------- host side


def _preprocess(edge_index, n_pad, npc, nt):
    """Sort/bin edges by dst, pack per-core edge-chunk metadata.

    Returns (K, per-core list of dicts) where every packed array has an
    identical shape across cores (SPMD requirement).
    """
    src = np.ascontiguousarray(edge_index[0]).astype(np.int64)
    dst = np.ascontiguousarray(edge_index[1]).astype(np.int64)

    deg = 1.0 + np.bincount(dst, minlength=n_pad).astype(np.float64)
    dis = 1.0 / np.sqrt(deg)
    norm = (dis[src] * dis[dst]).astype(np.float32)
    invdeg = (1.0 / deg).astype(np.float32)

    core = dst // npc
    percore = []
    for r in range(NCORES):
        m = core == r
        d_r = dst[m]
        o = np.argsort(d_r, kind="stable")
        s_s = src[m][o]
        d_s = d_r[o]
        w_s = norm[m][o]
        tid = (d_s - r * npc) >> 7
        cnt = np.bincount(tid, minlength=nt)
        starts = np.concatenate([[0], np.cumsum(cnt)])
        pos = np.arange(len(d_s)) - starts[tid]
        percore.append((s_s, d_s, w_s, tid, pos, cnt))

    kmax = max(int(math.ceil(pc[5].max() / P)) for pc in percore)
    kmax = max(kmax, 1)

    out = []
    for r in range(NCORES):
        s_s, d_s, w_s, tid, pos, cnt = percore[r]
        si = np.zeros((nt, kmax * P), np.int32)
        dl = np.zeros((nt, kmax * P), np.float32)
        nm = np.zeros((nt, kmax * P), np.float32)
        si[tid, pos] = s_s.astype(np.int32)
        dl[tid, pos] = (d_s % P).astype(np.float32)
        nm[tid, pos] = w_s

        def tr(a):
            return np.ascontiguousarray(
                a.reshape(nt, kmax, P).transpose(2, 0, 1).reshape(P, nt * kmax)
            )

        iv = np.ascontiguousarray(
            invdeg[r * npc:(r + 1) * npc].astype(np.float32).reshape(nt, P).T
        )
        out.append(
            dict(
                src_idx=tr(si),
                dstloc=tr(dl).astype(BF16),
                norm=tr(nm).astype(BF16),
                invdeg=iv,
            )
        )
    return kmax, out


# ---------------------------------------------------------------- device side


def _build_program(npc, nt, kk, in_dim, f, ncores):
    """Build the 3-layer GCN SPMD program. Returns compiled Bacc."""
    from concourse import bass, mybir, tile, bacc
    from concourse.masks import make_identity

    bf = mybir.dt.bfloat16
    f32 = mybir.dt.float32
    AF = mybir.ActivationFunctionType
    Alu = mybir.AluOpType

    nc = bacc.Bacc("TRN2", target_bir_lowering=False, debug=False,
                   enable_asserts=True, num_devices=ncores)

    # ---- I/O
    x_in = nc.dram_tensor("x", [npc, in_dim], bf, kind="ExternalInput")
    w_in = [
        nc.dram_tensor("w1", [in_dim, f], bf, kind="ExternalInput"),
        nc.dram_tensor("w2", [f, f], bf, kind="ExternalInput"),
        nc.dram_tensor("w3", [f, f], bf, kind="ExternalInput"),
    ]
    srcidx_in = nc.dram_tensor("src_idx", [P, nt * kk], mybir.dt.int32,
                               kind="ExternalInput")
    dstloc_in = nc.dram_tensor("dstloc", [P, nt * kk], bf, kind="ExternalInput")
    norm_in = nc.dram_tensor("norm", [P, nt * kk], bf, kind="ExternalInput")
    invdeg_in = nc.dram_tensor("invdeg", [P, nt], f32, kind="ExternalInput")
    y_out = nc.dram_tensor("y", [npc, f], f32, kind="ExternalOutput")

    # ---- internal DRAM
    ag_in = nc.dram_tensor("ag_in", [npc, f], bf, kind="Internal")
    h_full = nc.dram_tensor("h_full", [npc * ncores, f], bf, kind="Internal")
    zd_loc = nc.dram_tensor("zd_loc", [npc, f], bf, kind="Internal")
    out_loc = nc.dram_tensor("out_loc", [npc, f], bf, kind="Internal")

    with tile.TileContext(nc) as tc:
        with tc.tile_pool(name="consts", bufs=1) as cpool, \
             tc.tile_pool(name="work", bufs=3) as wpool, \
             tc.tile_pool(name="gather", bufs=3) as gpool, \
             tc.tile_pool(name="stats", bufs=8) as spool, \
             tc.tile_pool(name="psA", bufs=2, space="PSUM") as psA, \
             tc.tile_pool(name="psB", bufs=4, space="PSUM") as psB:

            # ---- persistent constants in SBUF
            srcidx_sb = cpool.tile([P, nt * kk], mybir.dt.int32, tag="srcidx")
            nc.sync.dma_start(srcidx_sb[:], srcidx_in[:])
            dstloc_sb = cpool.tile([P, nt * kk], bf, tag="dstloc")
            nc.sync.dma_start(dstloc_sb[:], dstloc_in[:])
            norm_sb = cpool.tile([P, nt * kk], bf, tag="norm")
            nc.sync.dma_start(norm_sb[:], norm_in[:])
            invdeg_sb = cpool.tile([P, nt], f32, tag="invdeg")
            nc.sync.dma_start(invdeg_sb[:], invdeg_in[:])

            # weight tiles: w[l] as list of [P, f] k-blocks
            w_sb = []
            for l, w in enumerate(w_in):
                kin = w.shape[0]
                blocks = []
                for b in range(kin // P):
                    t = cpool.tile([P, f], bf, tag=f"w{l}_{b}")
                    nc.sync.dma_start(t[:], w[b * P:(b + 1) * P, :])
                    blocks.append(t)
                w_sb.append(blocks)

            # iota along free (value = position within 128-block), bf16 exact
            iota_sb = cpool.tile([P, kk * P], bf, tag="iota")
            nc.gpsimd.iota(iota_sb[:], pattern=[[0, kk], [1, P]], base=0,
                           channel_multiplier=0,
                           allow_small_or_imprecise_dtypes=True)
            # identity for the self-loop matmul
            ident_sb = cpool.tile([P, P], bf, tag="ident")
            make_identity(nc, ident_sb[:])
            # LN epsilon as a per-partition scalar for the Sqrt bias
            eps_sb = cpool.tile([P, 1], f32, tag="eps")
            nc.gpsimd.memset(eps_sb[:], LN_EPS)

            layer_in = [x_in, out_loc, out_loc]
            kin_l = [in_dim, f, f]

            for l in range(3):
                # ---------- phase A: z = input @ W (local rows)
                for t in range(nt):
                    zp = psA.tile([P, f], f32, tag="zpsum")
                    nkb = kin_l[l] // P
                    for kb in range(nkb):
                        xT = wpool.tile([P, P], bf, tag="xT")
                        nc.sync.dma_start(
                            xT[:],
                            layer_in[l][t * P:(t + 1) * P, kb * P:(kb + 1) * P],
                            transpose=True,
                        )
                        nc.tensor.matmul(out=zp[:], lhsT=xT[:], rhs=w_sb[l][kb][:],
                                         start=(kb == 0), stop=(kb == nkb - 1))
                    zt = wpool.tile([P, f], bf, tag="zt")
                    nc.scalar.copy(zt[:], zp[:])
                    nc.sync.dma_start(ag_in[t * P:(t + 1) * P, :], zt[:])
                    zdt = wpool.tile([P, f], bf, tag="zdt")
                    nc.vector.tensor_scalar(zdt[:], zp[:], invdeg_sb[:, t:t + 1],
                                            None, op0=Alu.mult)
                    nc.sync.dma_start(zd_loc[t * P:(t + 1) * P, :], zdt[:])

                # ---------- AllGather z -> h_full
                nc.gpsimd.collective_compute(
                    "AllGather",
                    Alu.bypass,
                    replica_groups=[list(range(ncores))],
                    ins=[ag_in[:]],
                    outs=[h_full[:]],
                )

                # ---------- phase B: aggregate + self + relu (+ LN)
                for t in range(nt):
                    g_t = gpool.tile([P, kk * f], bf, tag="gt")
                    nc.gpsimd.indirect_dma_start(
                        out=g_t[:],
                        out_offset=None,
                        in_=h_full[:],
                        in_offset=bass.IndirectOffsetOnAxis(
                            ap=srcidx_sb[:, t * kk:(t + 1) * kk], axis=0),
                    )
                    s_t = gpool.tile([P, kk * P], bf, tag="st")
                    dl3 = dstloc_sb[:, t * kk:(t + 1) * kk].to_broadcast(
                        [P, kk, P])
                    nm3 = norm_sb[:, t * kk:(t + 1) * kk].to_broadcast(
                        [P, kk, P])
                    s3 = s_t[:].rearrange("p (k q) -> p k q", q=P)
                    i3 = iota_sb[:].rearrange("p (k q) -> p k q", q=P)
                    nc.vector.tensor_tensor(s3, i3, dl3, op=Alu.is_equal)
                    nc.vector.tensor_tensor(s3, s3, nm3, op=Alu.mult)

                    ap = psB.tile([P, f], f32, tag="agg")
                    for j in range(kk):
                        nc.tensor.matmul(out=ap[:],
                                         lhsT=s_t[:, j * P:(j + 1) * P],
                                         rhs=g_t[:, j * f:(j + 1) * f],
                                         start=(j == 0), stop=False)
                    zdt = wpool.tile([P, f], bf, tag="zdl")
                    nc.sync.dma_start(zdt[:], zd_loc[t * P:(t + 1) * P, :])
                    nc.tensor.matmul(out=ap[:], lhsT=ident_sb[:], rhs=zdt[:],
                                     start=False, stop=True)

                    if l < 2:
                        # relu + layernorm (g=1, be=0 for this model)
                        vr = wpool.tile([P, f], f32, tag="vr")
                        musum = spool.tile([P, 1], f32, tag="musum")
                        nc.scalar.activation(vr[:], ap[:], AF.Relu,
                                             accum_out=musum[:])
                        mu = spool.tile([P, 1], f32, tag="mu")
                        nc.scalar.activation(mu[:], musum[:], AF.Copy,
                                             scale=1.0 / f)
                        d = wpool.tile([P, f], f32, tag="d")
                        nc.vector.tensor_scalar(d[:], vr[:], mu[:], None,
                                                op0=Alu.subtract)
                        sq = wpool.tile([P, f], f32, tag="sq")
                        varsum = spool.tile([P, 1], f32, tag="varsum")
                        nc.scalar.activation(sq[:], d[:], AF.Square,
                                             accum_out=varsum[:])
                        std = spool.tile([P, 1], f32, tag="std")
                        nc.scalar.activation(std[:], varsum[:], AF.Sqrt,
                                             bias=eps_sb[:], scale=1.0 / f)
                        rs = spool.tile([P, 1], f32, tag="rs")
                        nc.vector.reciprocal(rs[:], std[:])
                        y_t = wpool.tile([P, f], bf, tag="yt")
                        nc.vector.tensor_scalar(y_t[:], d[:], rs[:], None,
                                                op0=Alu.mult)
                        nc.sync.dma_start(out_loc[t * P:(t + 1) * P, :], y_t[:])
                    else:
                        y_t = wpool.tile([P, f], f32, tag="yf")
                        nc.scalar.activation(y_t[:], ap[:], AF.Relu)
                        nc.sync.dma_start(y_out[t * P:(t + 1) * P, :], y_t[:])

    nc.compile()
    return nc


# ---------------------------------------------------------------- entry point


def run_gcn(x, edge_index, Ws, n, e, in_dim, f, ncores=NCORES, trace=False):
    """Generic runner used by kernel() and by the mini test."""
    from concourse import bass_utils

    npc = int(math.ceil(n / (ncores * P))) * P
    n_pad = npc * ncores
    nt = npc // P

    kmax, meta = _preprocess(edge_index, n_pad, npc, nt)

    x_pad = np.zeros((n_pad, in_dim), np.float32)
    x_pad[:n] = np.asarray(x, np.float32)
    x_bf = x_pad.astype(BF16)
    w_bf = [np.asarray(w, np.float32).astype(BF16) for w in Ws]

    nc = _build_program(npc, nt, kmax, in_dim, f, ncores)

    in_maps = []
    for r in range(ncores):
        m = meta[r]
        in_maps.append({
            "x": x_bf[r * npc:(r + 1) * npc],
            "w1": w_bf[0], "w2": w_bf[1], "w3": w_bf[2],
            "src_idx": m["src_idx"],
            "dstloc": m["dstloc"],
            "norm": m["norm"],
            "invdeg": m["invdeg"],
        })

    res = bass_utils.run_bass_kernel_spmd(
        nc, in_maps, core_ids=list(range(ncores)), trace=trace)

    y = np.concatenate([res.results[r]["y"] for r in range(ncores)], axis=0)
    return y[:n], res


def kernel(x, edge_index, W1, b1, W2, b2, W3, b3, g1, be1, g2, be2):
    # b1..b3 are zeros and g/be are identity for this model; verified on host
    # (they fold away from the device program).
    assert not np.any(np.asarray(b1)) and not np.any(np.asarray(b2)) \
        and not np.any(np.asarray(b3))
    assert np.all(np.asarray(g1) == 1) and np.all(np.asarray(g2) == 1)
    assert not np.any(np.asarray(be1)) and not np.any(np.asarray(be2))

    y, _ = run_gcn(np.asarray(x), np.asarray(edge_index), [W1, W2, W3],
                   N, E, IN_DIM, F)
    return y


# revision 10
# speedup vs baseline: 37.1523x; 1.1024x over previous
"""3-layer GCN encoder on 8 Trainium2 NeuronCores (Bass/Tile).

Sharding: 1D node partition (contiguous ranges) across 8 cores.
Per layer: local z = input @ W, AllGather z -> full node-feature table
(bf16), then each core aggregates messages for its own nodes:
bulk dma_gather of source rows (edges binned by dst tile x src block),
one-hot S matrices built on VectorE, segment sums via TensorE matmuls
accumulating in PSUM, self-loop via an identity matmul, then
ReLU (+LayerNorm) fused on ScalarE/VectorE.
"""
import math

import numpy as np
import ml_dtypes

BF16 = ml_dtypes.bfloat16

# problem constants (hardcoded per contract)
N = 100000
E = 1600000
IN_DIM = 512
F = 256
LN_EPS = 1e-5
NCORES = 8
P = 128
BLOCKS = 4          # gather-table blocks (int16 index range)


# ---------------------------------------------------------------- host side


def _preprocess(edge_index, n_pad, npc, nt, tt):
    """Bin edges by (dst-core, dst-tile, src-block); pack chunk metadata.

    Returns (K_b, per-core dict of packed arrays). All shapes identical
    across cores (SPMD requirement).
    """
    src = np.ascontiguousarray(edge_index[0]).astype(np.int64)
    dst = np.ascontiguousarray(edge_index[1]).astype(np.int64)
    bs = n_pad // BLOCKS

    deg = 1.0 + np.bincount(dst, minlength=n_pad).astype(np.float64)
    dis = 1.0 / np.sqrt(deg)
    norm = (dis[src] * dis[dst]).astype(np.float32)
    invdeg = (1.0 / deg).astype(np.float32)

    core = dst // npc
    parts = []
    kmax = 1
    for r in range(NCORES):
        m = core == r
        s_r, d_r, w_r = src[m], dst[m], norm[m]
        tid = (d_r - r * npc) >> 7
        blk = s_r // bs
        key = tid * BLOCKS + blk
        o = np.argsort(key, kind="stable")
        s_s, d_s, w_s, key_s = s_r[o], d_r[o], w_r[o], key[o]
        cnt = np.bincount(key_s, minlength=nt * BLOCKS)
        starts = np.concatenate([[0], np.cumsum(cnt)])
        pos = np.arange(len(key_s)) - starts[key_s]
        kmax = max(kmax, int(math.ceil(cnt.max() / P)))
        parts.append((s_s, d_s, w_s, key_s, pos))

    kb = kmax
    kt = BLOCKS * kb
    ng = nt // tt

    out = []
    for r in range(NCORES):
        s_s, d_s, w_s, key_s, pos = parts[r]
        # slot arrays [nt*BLOCKS, kb*128]
        si = np.zeros((nt * BLOCKS, kb * P), np.int64)
        dl = np.zeros((nt * BLOCKS, kb * P), np.float32)
        nm = np.zeros((nt * BLOCKS, kb * P), np.float32)
        si[key_s, pos] = s_s
        dl[key_s, pos] = (d_s % P).astype(np.float32)
        nm[key_s, pos] = w_s

        # dstloc / norm: [128, nt*kt], slot (p, t*kt + b*kb + j) = edge slot
        def tr(a):
            return np.ascontiguousarray(
                a.reshape(nt, BLOCKS, kb, P)
                .transpose(3, 0, 1, 2)
                .reshape(P, nt * kt)
            )

        # gather idx stream: per (group, block) call, flat order
        # i = (tl*kb + j)*128 + p ; value = src - b*bs (pads -> 0)
        blk_base = (np.arange(nt * BLOCKS) % BLOCKS) * bs
        si_rel = (si - blk_base[:, None]).astype(np.int64)
        si_rel[nm == 0] = 0  # pads gather block row 0
        assert si_rel.min() >= 0 and si_rel.max() < bs <= 32768
        si4 = si_rel.reshape(nt, BLOCKS, kb * P)
        cols = []
        for g in range(ng):
            for b in range(BLOCKS):
                flat = si4[g * tt:(g + 1) * tt, b, :].reshape(-1)
                cols.append(flat.reshape(-1, 16).T.astype(np.int16))
        idxs = np.tile(np.concatenate(cols, axis=1), (NCORES, 1))

        iv = np.ascontiguousarray(
            invdeg[r * npc:(r + 1) * npc].astype(np.float32).reshape(nt, P).T
        )
        out.append(
            dict(
                idxs=np.ascontiguousarray(idxs),
                dstloc=tr(dl).astype(BF16),
                norm=tr(nm).astype(BF16),
                invdeg=iv,
            )
        )
    return kb, out


# ---------------------------------------------------------------- device side


def _build_program(npc, nt, tt, kb, in_dim, f, ncores, debug=False):
    """Build the 3-layer GCN SPMD program. Returns compiled Bacc."""
    from concourse import bass, mybir, tile, bacc
    from concourse.masks import make_identity
    from concourse.library_config import mlp

    bf = mybir.dt.bfloat16
    f32 = mybir.dt.float32
    AF = mybir.ActivationFunctionType
    Alu = mybir.AluOpType

    kt = BLOCKS * kb
    ng = nt // tt
    n_pad = npc * ncores
    bs = n_pad // BLOCKS
    nidx_call = tt * kb * P
    l16 = nidx_call // 16
    idx_cols = ng * BLOCKS * l16

    nc = bacc.Bacc("TRN2", target_bir_lowering=False, debug=False,
                   enable_asserts=True, num_devices=ncores)

    # ---- I/O
    x_in = nc.dram_tensor("x", [npc, in_dim], bf, kind="ExternalInput")
    w_in = [
        nc.dram_tensor("w1", [in_dim, f], bf, kind="ExternalInput"),
        nc.dram_tensor("w2", [f, f], bf, kind="ExternalInput"),
        nc.dram_tensor("w3", [f, f], bf, kind="ExternalInput"),
    ]
    idxs_in = nc.dram_tensor("idxs", [P, idx_cols], mybir.dt.int16,
                             kind="ExternalInput")
    dstloc_in = nc.dram_tensor("dstloc", [P, nt * kt], bf, kind="ExternalInput")
    norm_in = nc.dram_tensor("norm", [P, nt * kt], bf, kind="ExternalInput")
    invdeg_in = nc.dram_tensor("invdeg", [P, nt], f32, kind="ExternalInput")
    y_out = nc.dram_tensor("y", [npc, f], f32, kind="ExternalOutput")
    if debug:
        dbg_z = nc.dram_tensor("dbg_z", [npc, f], bf, kind="ExternalOutput")
        dbg_zd = nc.dram_tensor("dbg_zd", [npc, f], bf, kind="ExternalOutput")
        dbg_h = nc.dram_tensor("dbg_h", [n_pad, f], bf, kind="ExternalOutput")
        dbg_agg = nc.dram_tensor("dbg_agg", [npc, f], f32,
                                 kind="ExternalOutput")

    # ---- internal DRAM
    ag_in = nc.dram_tensor("ag_in", [npc, f], bf, kind="Internal")
    h_full = nc.dram_tensor("h_full", [n_pad, f], bf, kind="Internal")
    zd_loc = nc.dram_tensor("zd_loc", [npc, f], bf, kind="Internal")
    out_loc = nc.dram_tensor("out_loc", [npc, f], bf, kind="Internal")

    with tile.TileContext(nc) as tc:
        with tc.tile_pool(name="consts", bufs=1) as cpool, \
             tc.tile_pool(name="work", bufs=3) as wpool, \
             tc.tile_pool(name="gather", bufs=2) as gpool, \
             tc.tile_pool(name="smat", bufs=3) as spool2, \
             tc.tile_pool(name="stats", bufs=8) as spool, \
             tc.tile_pool(name="psA", bufs=2, space="PSUM") as psA, \
             tc.tile_pool(name="psB", bufs=4, space="PSUM") as psB:

            # iota / identity first (gpsimd base ops), then the mlp library
            iota_sb = cpool.tile([P, kt * P], bf, tag="iota")
            nc.gpsimd.iota(iota_sb[:], pattern=[[0, kt], [1, P]], base=0,
                           channel_multiplier=0,
                           allow_small_or_imprecise_dtypes=True)
            ident_sb = cpool.tile([P, P], bf, tag="ident")
            make_identity(nc, ident_sb[:])
            eps_sb = cpool.tile([P, 1], f32, tag="eps")
            nc.gpsimd.memset(eps_sb[:], LN_EPS)
            nc.gpsimd.load_library(mlp)

            # ---- persistent constants in SBUF
            idxs_sb = cpool.tile([P, idx_cols], mybir.dt.int16, tag="idxs")
            nc.sync.dma_start(idxs_sb[:], idxs_in[:])
            dstloc_sb = cpool.tile([P, nt * kt], bf, tag="dstloc")
            nc.sync.dma_start(dstloc_sb[:], dstloc_in[:])
            norm_sb = cpool.tile([P, nt * kt], bf, tag="norm")
            nc.sync.dma_start(norm_sb[:], norm_in[:])
            invdeg_sb = cpool.tile([P, nt], f32, tag="invdeg")
            nc.sync.dma_start(invdeg_sb[:], invdeg_in[:])

            w_sb = []
            for l, w in enumerate(w_in):
                kin = w.shape[0]
                blocks = []
                for b in range(kin // P):
                    t = cpool.tile([P, f], bf, tag=f"w{l}_{b}")
                    nc.sync.dma_start(t[:], w[b * P:(b + 1) * P, :])
                    blocks.append(t)
                w_sb.append(blocks)

            layer_in = [x_in, out_loc, out_loc]
            kin_l = [in_dim, f, f]

            for l in range(3):
                # ---------- phase A: z = input @ W (local rows)
                for t in range(nt):
                    zp = psA.tile([P, f], f32, tag="zpsum")
                    nkb = kin_l[l] // P
                    for kbi in range(nkb):
                        xT = wpool.tile([P, P], bf, tag="xT")
                        nc.sync.dma_start(
                            xT[:],
                            layer_in[l][t * P:(t + 1) * P,
                                        kbi * P:(kbi + 1) * P],
                            transpose=True,
                        )
                        nc.tensor.matmul(out=zp[:], lhsT=xT[:],
                                         rhs=w_sb[l][kbi][:],
                                         start=(kbi == 0),
                                         stop=(kbi == nkb - 1))
                    zt = wpool.tile([P, f], bf, tag="zt")
                    nc.scalar.copy(zt[:], zp[:])
                    nc.sync.dma_start(ag_in[t * P:(t + 1) * P, :], zt[:])
                    zdt = wpool.tile([P, f], bf, tag="zdt")
                    nc.vector.tensor_scalar(zdt[:], zp[:],
                                            invdeg_sb[:, t:t + 1],
                                            None, op0=Alu.mult)
                    nc.sync.dma_start(zd_loc[t * P:(t + 1) * P, :], zdt[:])
                    if debug and l == 0:
                        nc.sync.dma_start(dbg_z[t * P:(t + 1) * P, :], zt[:])
                        nc.sync.dma_start(dbg_zd[t * P:(t + 1) * P, :], zdt[:])

                # ---------- AllGather z -> h_full
                nc.gpsimd.collective_compute(
                    "AllGather",
                    Alu.bypass,
                    replica_groups=[list(range(ncores))],
                    ins=[ag_in[:]],
                    outs=[h_full[:]],
                )

                if debug and l == 0:
                    for t in range(nt * ncores):
                        ht = wpool.tile([P, f], bf, tag="dbgh")
                        nc.sync.dma_start(ht[:], h_full[t * P:(t + 1) * P, :])
                        nc.sync.dma_start(dbg_h[t * P:(t + 1) * P, :], ht[:])

                # ---------- phase B: aggregate + self + relu (+ LN)
                for g in range(ng):
                    gts = []
                    for b in range(BLOCKS):
                        gt = gpool.tile([P, tt * kb, f], bf, tag=f"g{b}")
                        co = (g * BLOCKS + b) * l16
                        nc.gpsimd.dma_gather(
                            gt[:], h_full[b * bs:(b + 1) * bs, :],
                            idxs_sb[:, co:co + l16],
                            nidx_call, nidx_call, f)
                        gts.append(gt)

                    for tl in range(tt):
                        t = g * tt + tl
                        s_t = spool2.tile([P, kt * P], bf, tag="st")
                        dl3 = dstloc_sb[:, t * kt:(t + 1) * kt].to_broadcast(
                            [P, kt, P])
                        nm3 = norm_sb[:, t * kt:(t + 1) * kt].to_broadcast(
                            [P, kt, P])
                        s3 = s_t[:].rearrange("p (k q) -> p k q", q=P)
                        i3 = iota_sb[:].rearrange("p (k q) -> p k q", q=P)
                        nc.vector.tensor_tensor(s3, i3, dl3, op=Alu.is_equal)
                        nc.vector.tensor_tensor(s3, s3, nm3, op=Alu.mult)

                        ap = psB.tile([P, f], f32, tag="agg")
                        mm = 0
                        for b in range(BLOCKS):
                            for j in range(kb):
                                q = b * kb + j
                                nc.tensor.matmul(
                                    out=ap[:],
                                    lhsT=s_t[:, q * P:(q + 1) * P],
                                    rhs=gts[b][:, tl * kb + j, :],
                                    start=(mm == 0), stop=False)
                                mm += 1
                        zdt = wpool.tile([P, f], bf, tag="zdl")
                        nc.sync.dma_start(zdt[:], zd_loc[t * P:(t + 1) * P, :])
                        nc.tensor.matmul(out=ap[:], lhsT=ident_sb[:],
                                         rhs=zdt[:], start=False, stop=True)
                        if debug and l == 0:
                            at = wpool.tile([P, f], f32, tag="dbga")
                            nc.vector.tensor_copy(at[:], ap[:])
                            nc.sync.dma_start(
                                dbg_agg[t * P:(t + 1) * P, :], at[:])

                        if l < 2:
                            vr = wpool.tile([P, f], f32, tag="vr")
                            musum = spool.tile([P, 1], f32, tag="musum")
                            nc.scalar.activation(vr[:], ap[:], AF.Relu,
                                                 accum_out=musum[:])
                            mu = spool.tile([P, 1], f32, tag="mu")
                            nc.scalar.activation(mu[:], musum[:], AF.Copy,
                                                 scale=1.0 / f)
                            d = wpool.tile([P, f], f32, tag="d")
                            nc.vector.tensor_scalar(d[:], vr[:], mu[:], None,
                                                    op0=Alu.subtract)
                            sq = wpool.tile([P, f], f32, tag="sq")
                            varsum = spool.tile([P, 1], f32, tag="varsum")
                            nc.scalar.activation(sq[:], d[:], AF.Square,
                                                 accum_out=varsum[:])
                            std = spool.tile([P, 1], f32, tag="std")
                            nc.scalar.activation(std[:], varsum[:], AF.Sqrt,
                                                 bias=eps_sb[:], scale=1.0 / f)
                            rs = spool.tile([P, 1], f32, tag="rs")
                            nc.vector.reciprocal(rs[:], std[:])
                            y_t = wpool.tile([P, f], bf, tag="yt")
                            nc.vector.tensor_scalar(y_t[:], d[:], rs[:], None,
                                                    op0=Alu.mult)
                            nc.sync.dma_start(out_loc[t * P:(t + 1) * P, :],
                                              y_t[:])
                        else:
                            y_t = wpool.tile([P, f], f32, tag="yf")
                            nc.scalar.activation(y_t[:], ap[:], AF.Relu)
                            nc.sync.dma_start(y_out[t * P:(t + 1) * P, :],
                                              y_t[:])

    nc.compile()
    return nc


# ---------------------------------------------------------------- entry point


def run_gcn(x, edge_index, Ws, n, e, in_dim, f, ncores=NCORES, tt=7,
            trace=False, debug=False):
    """Generic runner used by kernel() and by the mini test."""
    from concourse import bass_utils

    npc = int(math.ceil(n / (ncores * P))) * P
    n_pad = npc * ncores
    nt = npc // P
    while nt % tt:
        tt -= 1

    kb, meta = _preprocess(edge_index, n_pad, npc, nt, tt)

    x_pad = np.zeros((n_pad, in_dim), np.float32)
    x_pad[:n] = np.asarray(x, np.float32)
    x_bf = x_pad.astype(BF16)
    w_bf = [np.asarray(w, np.float32).astype(BF16) for w in Ws]

    nc = _build_program(npc, nt, tt, kb, in_dim, f, ncores, debug=debug)

    in_maps = []
    for r in range(ncores):
        m = meta[r]
        in_maps.append({
            "x": x_bf[r * npc:(r + 1) * npc],
            "w1": w_bf[0], "w2": w_bf[1], "w3": w_bf[2],
            "idxs": m["idxs"],
            "dstloc": m["dstloc"],
            "norm": m["norm"],
            "invdeg": m["invdeg"],
        })

    res = bass_utils.run_bass_kernel_spmd(
        nc, in_maps, core_ids=list(range(ncores)), trace=trace)

    y = np.concatenate([res.results[r]["y"] for r in range(ncores)], axis=0)
    return y[:n], res


def kernel(x, edge_index, W1, b1, W2, b2, W3, b3, g1, be1, g2, be2):
    # b1..b3 are zeros and g/be are identity for this model; verified on host
    # (they fold away from the device program).
    assert not np.any(np.asarray(b1)) and not np.any(np.asarray(b2)) \
        and not np.any(np.asarray(b3))
    assert np.all(np.asarray(g1) == 1) and np.all(np.asarray(g2) == 1)
    assert not np.any(np.asarray(be1)) and not np.any(np.asarray(be2))

    y, _ = run_gcn(np.asarray(x), np.asarray(edge_index), [W1, W2, W3],
                   N, E, IN_DIM, F)
    return y
